# revision 2
# baseline (speedup 1.0000x reference)
"""Trainium2 Bass kernel for nn_ALNet (adaptive linear network forward).

Math: vals = x @ W + b  ([65536,256] @ [256,128] + [128]), then a 7-level
alternating min/max pairwise tree over the 128 leaf columns -> [B, 1].

v2 strategy (8 NeuronCores, data-parallel over batch; per-core shard 8192):
  Fully pipelined 1024-col units.  The three DMA streams are separated so
  nothing FIFO-blocks:
    - gpsimd (SWDGE): x K-half-0 chunk loads
    - scalar (HWDGE qAct): W+bias first, then x K-half-1 chunk loads
    - sync   (HWDGE qSP): ONLY the XBAR transposes + final out store, so the
      first transpose starts as soon as unit 0 is evicted (~12us) instead of
      queuing behind 2 MiB of x loads (~19us).
  Per 1024-col unit: 4 matmuls (2 K-halves x 2 psum banks, W-stationary,
  ACT-bias rides the eviction), evict alternating ACT/DVE, then per-span
  XBAR transpose -> L1 min on DVE -> levels 2..7 on the free dim.
  Warmup garbage matmuls keep the PE HAM clock at 2.4 GHz before real data.
"""

import numpy as np

try:
    import concourse.bass as bass
except ImportError:  # pragma: no cover
    import sys

    sys.path.insert(0, "/opt/trn_rl_repo")
    import concourse.bass as bass

import concourse.mybir as mybir
import concourse.tile as tile
from concourse import bacc
from concourse.bass_utils import run_bass_kernel_spmd

F32 = mybir.dt.float32
F16 = mybir.dt.float16

B, F, NL = 65536, 256, 128
NCORES = 8
BS = B // NCORES  # 8192 batch rows per core

UNIT = 1024  # psum-eviction granularity (2 banks)
NUNIT = BS // UNIT  # 8
# x DMA chunks (cols): bigger early for throughput, small last for the tail
DMA_CHUNKS = [(0, 2048), (2048, 2048), (4096, 2048), (6144, 1024), (7168, 1024)]
# transpose spans (col0, len)
T_SPANS = [(0, 2048), (2048, 2048), (4096, 2048), (6144, 1024), (7168, 1024)]
# tree spans (col0, len) and which engine runs levels 2..7
TREE_SPANS = [(0, 2048), (2048, 2048), (4096, 2048), (6144, 2048)]

# Tree ops, deepest level first (palindrome list: min,max,min,max,min,max,min)
_TREE_OPS = [
    mybir.AluOpType.min if i % 2 == 0 else mybir.AluOpType.max for i in range(7)
]


def _bitrev7_perm() -> np.ndarray:
    perm = np.zeros(NL, dtype=np.int64)
    for p in range(NL):
        r = 0
        for k in range(7):
            r |= ((p >> k) & 1) << (6 - k)
        perm[p] = r
    return perm


def build_nc(bs: int = BS):
    nc = bacc.Bacc(None)
    x0d = nc.declare_dram_parameter("x0", [128, bs], F16, isOutput=False)
    x1d = nc.declare_dram_parameter("x1", [128, bs], F16, isOutput=False)
    wbd = nc.declare_dram_parameter("wb", [128, 257], F16, isOutput=False)
    ncols = bs // 128  # 64
    out = nc.declare_dram_parameter("out", [128, ncols], F32, isOutput=True)

    ACT_EVICT = {0, 2, 4, 6, 7}  # eviction on ACT for these units; DVE rest

    with tile.TileContext(nc, pool_alloc_mode="queue") as tc:
        with (
            tc.tile_pool(name="xin", bufs=1) as xpool,
            tc.tile_pool(name="psum", bufs=4, space=bass.MemorySpace.PSUM) as ppool,
            tc.tile_pool(name="sb", bufs=1) as spool,
        ):
            x0 = xpool.tile([128, bs], F16, tag="x0")
            x1 = xpool.tile([128, bs], F16, tag="x1")
            wb = xpool.tile([128, 257], F16, tag="wb")

            # W + bias first on the scalar ring (small, lands fast)
            nc.scalar.dma_start(out=wb[:], in_=wbd[:])
            # x streams: K-half-0 via gpsimd SWDGE, K-half-1 via scalar HWDGE
            for col0, ln in DMA_CHUNKS:
                nc.gpsimd.dma_start(
                    out=x0[:, col0 : col0 + ln], in_=x0d[:, col0 : col0 + ln]
                )
            for col0, ln in DMA_CHUNKS:
                nc.scalar.dma_start(
                    out=x1[:, col0 : col0 + ln], in_=x1d[:, col0 : col0 + ln]
                )

            w0t = wb[:, 0:128]
            w1t = wb[:, 128:256]
            bch = wb[:, 256:257]
            bcf = spool.tile([128, 1], F32, tag="bcf")
            nc.vector.tensor_copy(bcf[:], bch)
            bct = bcf[:]

            # flat SBUF intermediates
            vb = spool.tile([128, bs], F16, tag="vb")  # [leaf, batch]
            vt = spool.tile([128, bs], F16, tag="vt")  # [batch_p, blk*128]
            l1 = spool.tile([128, bs // 2], F16, tag="l1")
            lvl_tiles = []
            w = 32
            n = bs // 4
            while w >= 2:
                lvl_tiles.append(
                    spool.tile([128, n], F16, tag=f"lv{w}", name=f"lv{w}")
                )
                w //= 2
                n //= 2
            ost = spool.tile([128, ncols], F32, tag="ost")

            # PE p-state warmup: garbage matmuls with no input deps keep the
            # PE streaming from the preamble until real x data lands
            garb = spool.tile([128, 512], F16, tag="garb")
            nc.vector.memset(garb[:], 0.0)
            pss = {}
            for u in range(NUNIT):
                pss[u] = ppool.tile([128, UNIT], F32, tag="ps", name=f"ps_{u}")
            for i in range(10):
                nc.tensor.matmul(
                    pss[0][:, 0:512], garb[:, 0:128], garb[:],
                    start=True, stop=True,
                )

            def evict(u):
                col0 = u * UNIT
                slc = slice(col0, col0 + UNIT)
                if u in ACT_EVICT:
                    nc.scalar.activation(
                        vb[:, slc], pss[u][:],
                        mybir.ActivationFunctionType.Identity,
                        bias=bct, scale=1.0,
                    )
                else:
                    nc.vector.tensor_scalar(
                        out=vb[:, slc], in0=pss[u][:], scalar1=bct,
                        scalar2=None, op0=mybir.AluOpType.add,
                    )

            def tr_l1(col0, ln):
                # NOTE: XBAR transposes must never overlap in time (shared
                # hw resource) -- keep them all on one queue (sync)
                nc.sync.dma_start(
                    out=vt[:, col0 : col0 + ln].rearrange(
                        "p (blk l) -> p blk l", l=128
                    ),
                    in_=vb[:, col0 : col0 + ln],
                    transpose=True,
                )
                rr = vt[:, col0 : col0 + ln].rearrange(
                    "p (blk two h) -> p blk two h", two=2, h=64
                )
                nc.vector.tensor_tensor(
                    out=l1[:, col0 // 2 : (col0 + ln) // 2].rearrange(
                        "p (blk h) -> p blk h", h=64
                    ),
                    in0=rr[:, :, 0, :], in1=rr[:, :, 1, :], op=_TREE_OPS[0],
                )

            def tree(col0, ncols_span):
                # levels 2..7 for batch cols [col0, col0+ncols_span)
                cur = l1[:, col0 // 2 : (col0 + ncols_span) // 2]
                w = 32
                for lvl in range(1, 7):
                    r = cur.rearrange("p (blk two h) -> p blk two h", two=2, h=w)
                    if lvl < 6:
                        base = lvl_tiles[lvl - 1]
                        nxt = base[
                            :, (col0 // 128) * w : ((col0 + ncols_span) // 128) * w
                        ]
                        outap = nxt.rearrange("p (blk h) -> p blk h", h=w)
                    else:
                        nxt = None
                        outap = ost[
                            :, col0 // 128 : (col0 + ncols_span) // 128
                        ].rearrange("p (blk h) -> p blk h", h=1)
                    nc.vector.tensor_tensor(
                        out=outap, in0=r[:, :, 0, :], in1=r[:, :, 1, :],
                        op=_TREE_OPS[lvl],
                    )
                    cur = nxt
                    w //= 2

            # main pipeline over 1024-col units
            tspan_done = 0
            tree_done = 0
            for u in range(NUNIT):
                ps = pss[u]
                c0 = u * UNIT
                for bank in range(2):
                    nc.tensor.matmul(
                        ps[:, bass.ts(bank, 512)], w0t,
                        x0[:, c0 + bank * 512 : c0 + bank * 512 + 512],
                        start=True, stop=False,
                    )
                for bank in range(2):
                    nc.tensor.matmul(
                        ps[:, bass.ts(bank, 512)], w1t,
                        x1[:, c0 + bank * 512 : c0 + bank * 512 + 512],
                        start=False, stop=True,
                    )
                evict(u)
                # fire any transpose spans fully evicted now
                end = c0 + UNIT
                while tspan_done < len(T_SPANS):
                    tc0, tln = T_SPANS[tspan_done]
                    if tc0 + tln > end:
                        break
                    tr_l1(tc0, tln)
                    tspan_done += 1
                # fire tree spans whose L1 halves are done
                while tree_done < len(TREE_SPANS):
                    rc0, rln = TREE_SPANS[tree_done]
                    done_cols = sum(
                        ln for (cc, ln) in T_SPANS[:tspan_done]
                    )
                    if rc0 + rln > done_cols:
                        break
                    tree(rc0, rln)
                    tree_done += 1

            nc.sync.dma_start(out=out[:], in_=ost[:])

    nc.compile()
    return nc


_NC_CACHE: dict = {}


def _get_nc(bs=BS):
    if bs not in _NC_CACHE:
        _NC_CACHE[bs] = build_nc(bs)
    return _NC_CACHE[bs]


def prep_inputs(x: np.ndarray, W: np.ndarray, b: np.ndarray) -> list[dict]:
    perm = _bitrev7_perm()
    Wp = np.ascontiguousarray(W[:, perm]).astype(np.float16)
    bh = b[perm].astype(np.float16)
    x = np.asarray(x, dtype=np.float32)
    wb = np.zeros((128, 257), dtype=np.float16)
    wb[:, 0:128] = Wp[0:128, :]
    wb[:, 128:256] = Wp[128:256, :]
    wb[:, 256] = bh
    in_maps = []
    for i in range(NCORES):
        xi = x[i * BS : (i + 1) * BS, :].astype(np.float16)  # [8192, 256]
        xT = np.ascontiguousarray(xi.T)  # [256, 8192]
        in_maps.append(
            {
                "x0": np.ascontiguousarray(xT[0:128, :]),
                "x1": np.ascontiguousarray(xT[128:256, :]),
                "wb": wb,
            }
        )
    return in_maps


def gather_outputs(results: list[dict]) -> np.ndarray:
    shards = []
    for i in range(NCORES):
        o = np.asarray(results[i]["out"])  # [128, BS//128]; o[p, c] = row 128c+p
        shards.append(o.T.reshape(BS))
    return np.concatenate(shards).reshape(B, 1).astype(np.float32)


def _setup_tracing():
    """Install the antenv.axon_hooks NTFF-profile shim (missing from this
    image) and neuter the artifact upload so traced runs stay local."""
    import sys as _sys
    import types

    import concourse.bass_utils as bu

    bu.upload_artifacts = lambda tmpdir: tmpdir
    try:
        from antenv.axon_hooks import get_axon_ntff_profile_hook  # noqa: F401

        return
    except ImportError:
        pass
    import antenv

    m = types.ModuleType("antenv.axon_hooks")
    _state = {"hook": None}
    m.set_axon_ntff_profile_hook = lambda h: _state.__setitem__("hook", h)
    m.get_axon_ntff_profile_hook = lambda: _state["hook"]
    _sys.modules["antenv.axon_hooks"] = m
    antenv.axon_hooks = m
    try:
        from trn_agent_boot.trn_boot import _ntff_profile_via_ctypes

        hook = _ntff_profile_via_ctypes("/opt/axon/libaxon_pjrt.so")
        if hook is not None:
            m.set_axon_ntff_profile_hook(hook)
    except Exception as e:  # pragma: no cover
        print("ntff hook install failed:", e)


def run_on_hw(x, W, b, trace: bool = False, **kwargs):
    if trace:
        _setup_tracing()
    nc = _get_nc()
    in_maps = prep_inputs(np.asarray(x), np.asarray(W), np.asarray(b))
    return run_bass_kernel_spmd(
        nc, in_maps, core_ids=list(range(NCORES)), trace=trace, **kwargs
    )


def kernel(x: np.ndarray, W: np.ndarray, b: np.ndarray) -> np.ndarray:
    res = run_on_hw(x, W, b, trace=False)
    return gather_outputs(res.results)
